# revision 1
# baseline (speedup 1.0000x reference)
"""GQA attention block (B=2,T=2048,E=2048,H=16,KV=4) on 8 trn2 NeuronCores.

Sharding: core c -> batch b=c//4, kv-group g=c%4 (q-heads 4g..4g+3, kv head g).
Each core computes its 4 heads end-to-end plus the partial output projection
(Wo rows for its heads); host sums the 4 partials per batch and adds bias.

Device-side layout tricks (all cores run one identical program, data differs):
  - q/k are produced directly transposed [d, T] (d on partitions) so the
    scores matmul S^T = kT.T-block @ qT and the PV matmul need no transposes.
  - RoPE pair interleave removed by host-permuting Wq/Wk columns per head to
    [64 even | 64 odd]; the r/i half swap is one partition-swap SBUF DMA.
  - rope(q + bq) handled by adding bq (per-partition scalar) during PSUM
    evacuation, before the cos/sin multiply. bv folded into bo on host
    (attn rows sum to const after softmax). Softmax post-scale folded into Wo.
  - Softmax over the partition dim (S^T rows) via ones-vector matmuls on PE,
    reciprocal on DVE, and a K=1 broadcast matmul to divide O^T columns.
"""

import numpy as np

for _p in ("/opt/trn_rl_repo", "/root/.axon_site/_ro/trn_rl_repo"):
    import sys

    if _p not in sys.path:
        sys.path.insert(0, _p)

import ml_dtypes
from contextlib import ExitStack

import concourse.bass as bass
import concourse.mybir as mybir
import concourse.tile as tile
from concourse import bacc
from concourse.bass_utils import run_bass_kernel_spmd

F32 = mybir.dt.float32
BF16 = mybir.dt.bfloat16
F16 = mybir.dt.float16
T = 2048
E = 2048
HD = 128
NQH = 4          # q heads per core
SCALE = float(E) ** -0.5

_program = None
LAST_EXEC_NS = None


def _build_program():
    nc = bacc.Bacc("TRN2", target_bir_lowering=False, debug=False, num_devices=8)
    xT_d = nc.declare_dram_parameter("xT", [E, T], F16, isOutput=False)
    wq_d = nc.declare_dram_parameter("wq", [E, NQH * HD], F16, isOutput=False)
    wk_d = nc.declare_dram_parameter("wk", [E, HD], F16, isOutput=False)
    wv_d = nc.declare_dram_parameter("wv", [E, HD], F16, isOutput=False)
    wo_d = nc.declare_dram_parameter("wo", [NQH * HD, E], BF16, isOutput=False)
    ct_d = nc.declare_dram_parameter("ct", [HD, T], F32, isOutput=False)
    st_d = nc.declare_dram_parameter("st", [HD, T], F32, isOutput=False)
    mask_d = nc.declare_dram_parameter("mask", [4, HD, 512], BF16, isOutput=False)
    bq_d = nc.declare_dram_parameter("bq", [HD, NQH], F32, isOutput=False)
    bk_d = nc.declare_dram_parameter("bk", [HD, 1], F32, isOutput=False)
    out_d = nc.declare_dram_parameter("out", [T, E], F32, isOutput=True)

    KT = E // 128    # 16 k-tiles over embed
    NT = T // 128    # 16 tiles over time
    NC = T // 512    # 4 512-chunks over time

    with tile.TileContext(nc) as tc, ExitStack() as ctx:
        consts = ctx.enter_context(tc.tile_pool(name="consts", bufs=1))
        rope = ctx.enter_context(tc.tile_pool(name="rope", bufs=2))
        ptp = ctx.enter_context(tc.tile_pool(name="ptp", bufs=8))
        ptep = ctx.enter_context(tc.tile_pool(name="ptep", bufs=3))
        otp = ctx.enter_context(tc.tile_pool(name="otp", bufs=6))
        outp = ctx.enter_context(tc.tile_pool(name="outp", bufs=2))
        dvp = ctx.enter_context(tc.tile_pool(name="dvp", bufs=2))
        bip = ctx.enter_context(tc.tile_pool(name="bip", bufs=2))
        psA = ctx.enter_context(tc.tile_pool(name="psA", bufs=4, space=bass.MemorySpace.PSUM))
        psOT = ctx.enter_context(tc.tile_pool(name="psOT", bufs=2, space=bass.MemorySpace.PSUM))
        psD = ctx.enter_context(tc.tile_pool(name="psD", bufs=1, space=bass.MemorySpace.PSUM))
        psB = ctx.enter_context(tc.tile_pool(name="psB", bufs=1, space=bass.MemorySpace.PSUM))

        # ---- resident constants -------------------------------------------
        xt = []
        for k in range(KT):
            t_ = consts.tile([128, T], F16, tag=f"xt{k}")
            nc.sync.dma_start(t_[:], xT_d[k * 128:(k + 1) * 128, :])
            xt.append(t_)
        wq = []
        for k in range(KT):
            t_ = consts.tile([128, NQH * HD], F16, tag=f"wq{k}")
            nc.sync.dma_start(t_[:], wq_d[k * 128:(k + 1) * 128, :])
            wq.append(t_)
        wk = []
        wv = []
        for k in range(KT):
            t_ = consts.tile([128, HD], F16, tag=f"wk{k}")
            nc.sync.dma_start(t_[:], wk_d[k * 128:(k + 1) * 128, :])
            wk.append(t_)
            t_ = consts.tile([128, HD], F16, tag=f"wv{k}")
            nc.sync.dma_start(t_[:], wv_d[k * 128:(k + 1) * 128, :])
            wv.append(t_)
        wo = []
        for h in range(NQH):
            t_ = consts.tile([128, E], BF16, tag=f"wo{h}")
            nc.sync.dma_start(t_[:], wo_d[h * 128:(h + 1) * 128, :])
            wo.append(t_)
        ct = consts.tile([128, T], F32, tag="ct")
        nc.sync.dma_start(ct[:], ct_d[:])
        st = consts.tile([128, T], F32, tag="st")
        nc.sync.dma_start(st[:], st_d[:])
        msk = []
        for j in range(4):
            t_ = consts.tile([128, 512], BF16, tag=f"msk{j}")
            nc.sync.dma_start(t_[:], mask_d[j])
            msk.append(t_)
        bq_t = consts.tile([HD, NQH], F32, tag="bq")
        nc.sync.dma_start(bq_t[:], bq_d[:])
        bk_t = consts.tile([HD, 1], F32, tag="bk")
        nc.sync.dma_start(bk_t[:], bk_d[:])
        ones_col = consts.tile([128, 1], BF16, tag="onc")
        nc.vector.memset(ones_col[:], 1.0)
        ones_row = consts.tile([1, 128], F32, tag="onr")
        nc.vector.memset(ones_row[:], 1.0)

        qT = []
        for h in range(NQH):
            qT.append(consts.tile([128, T], BF16, tag=f"qT{h}", name=f"qT{h}"))
        kTt = consts.tile([128, T], BF16, tag="kT")
        vA = consts.tile([128, T], BF16, tag="vA")

        # ---- projections + rope -------------------------------------------
        def rope_chunk(ps, bias_ap, dst, col0):
            sl = slice(col0, col0 + 512)
            qsb = rope.tile([128, 512], F32, tag="qsb")
            nc.scalar.activation(
                qsb[:], ps[:], mybir.ActivationFunctionType.Identity, bias=bias_ap)
            qsw = rope.tile([128, 512], F32, tag="qsw")
            nc.sync.dma_start(qsw[0:64, :], qsb[64:128, :])
            nc.sync.dma_start(qsw[64:128, :], qsb[0:64, :])
            t1 = rope.tile([128, 512], F32, tag="t1")
            nc.vector.tensor_mul(t1[:], qsb[:], ct[:, sl])
            t2 = rope.tile([128, 512], F32, tag="t2")
            nc.vector.tensor_mul(t2[:], qsw[:], st[:, sl])
            nc.vector.tensor_add(dst[:, sl], t1[:], t2[:])

        for h in range(NQH):
            for c in range(NC):
                ps = psA.tile([128, 512], F32, tag="ps")
                for k in range(KT):
                    nc.tensor.matmul(
                        ps[:], wq[k][:, h * HD:(h + 1) * HD],
                        xt[k][:, c * 512:(c + 1) * 512],
                        start=(k == 0), stop=(k == KT - 1))
                rope_chunk(ps, bq_t[:, h:h + 1], qT[h], c * 512)
        for c in range(NC):
            ps = psA.tile([128, 512], F32, tag="ps")
            for k in range(KT):
                nc.tensor.matmul(
                    ps[:], wk[k][:], xt[k][:, c * 512:(c + 1) * 512],
                    start=(k == 0), stop=(k == KT - 1))
            rope_chunk(ps, bk_t[:, 0:1], kTt, c * 512)
        for tt in range(NT):
            ps = psA.tile([128, HD], F32, tag="ps")
            for k in range(KT):
                nc.tensor.matmul(
                    ps[:], xt[k][:, tt * 128:(tt + 1) * 128], wv[k][:],
                    start=(k == 0), stop=(k == KT - 1))
            nc.scalar.copy(vA[:, tt * 128:(tt + 1) * 128], ps[:])

        # ---- attention + output projection, per 512-query chunk -----------
        for qc in range(NC):
            ots = []
            for h in range(NQH):
                ntk = 4 * (qc + 1)
                psd = psD.tile([1, 512], F32, tag="psd")
                psot = psOT.tile([128, 512], F32, tag="psot")
                for tk in range(ntk):
                    pss = psA.tile([128, 512], F32, tag="ps")
                    nc.tensor.matmul(
                        pss[:], kTt[:, tk * 128:(tk + 1) * 128],
                        qT[h][:, qc * 512:(qc + 1) * 512],
                        start=True, stop=True)
                    pt = ptp.tile([128, 512], BF16, tag="pt")
                    if tk >= 4 * qc:
                        pte = ptep.tile([128, 512], BF16, tag="pte")
                        nc.scalar.activation(
                            pte[:], pss[:], mybir.ActivationFunctionType.Exp)
                        nc.vector.tensor_mul(pt[:], pte[:], msk[tk - 4 * qc][:])
                    else:
                        nc.scalar.activation(
                            pt[:], pss[:], mybir.ActivationFunctionType.Exp)
                    nc.tensor.matmul(
                        psd[:], ones_col[:], pt[:],
                        start=(tk == 0), stop=(tk == ntk - 1))
                    nc.tensor.matmul(
                        psot[:], vA[:, tk * 128:(tk + 1) * 128], pt[:],
                        start=(tk == 0), stop=(tk == ntk - 1))
                dinv = dvp.tile([1, 512], F32, tag="dinv")
                nc.vector.reciprocal(dinv[:], psd[:])
                psb = psB.tile([128, 512], F32, tag="psb")
                nc.tensor.matmul(psb[:], ones_row[:], dinv[:], start=True, stop=True)
                binv = bip.tile([128, 512], F32, tag="binv")
                nc.scalar.copy(binv[:], psb[:])
                otn = otp.tile([128, 512], BF16, tag="otn")
                nc.vector.tensor_mul(otn[:], psot[:], binv[:])
                ots.append(otn)
            for i in range(4):
                for e in range(NC):
                    psf = psA.tile([128, 512], F32, tag="ps")
                    for h in range(NQH):
                        nc.tensor.matmul(
                            psf[:], ots[h][:, i * 128:(i + 1) * 128],
                            wo[h][:, e * 512:(e + 1) * 512],
                            start=(h == 0), stop=(h == NQH - 1))
                    osb = outp.tile([128, 512], F32, tag="osb")
                    nc.vector.tensor_copy(osb[:], psf[:])
                    nc.sync.dma_start(
                        out_d[(qc * 4 + i) * 128:(qc * 4 + i + 1) * 128,
                              e * 512:(e + 1) * 512],
                        osb[:])
    nc.compile()
    return nc


def _rope_tables():
    # quirk: freq exponent uses full n_embed then slices to head_dim//2
    freqs = 10000.0 ** (-(np.arange(0, E, 2, dtype=np.float64) / E))[:HD // 2]
    t = np.arange(T, dtype=np.float64)
    ang = np.outer(freqs, t)                      # [64, T]
    ct = np.empty((HD, T), np.float32)
    st = np.empty((HD, T), np.float32)
    ct[:64] = np.cos(ang)
    ct[64:] = np.cos(ang)
    st[:64] = -np.sin(ang)
    st[64:] = np.sin(ang)
    return ct, st


def kernel(x, Wq, bq, Wk, bk, Wv, bv, Wo, bo):
    global _program, LAST_EXEC_NS
    x = np.asarray(x, np.float32)
    Wq, bq = np.asarray(Wq, np.float32), np.asarray(bq, np.float32)
    Wk, bk = np.asarray(Wk, np.float32), np.asarray(bk, np.float32)
    Wv, bv = np.asarray(Wv, np.float32), np.asarray(bv, np.float32)
    Wo, bo = np.asarray(Wo, np.float32), np.asarray(bo, np.float32)
    bf = ml_dtypes.bfloat16

    if _program is None:
        _program = _build_program()

    perm = np.concatenate([np.arange(0, HD, 2), np.arange(1, HD, 2)])
    ct, st = _rope_tables()
    mask = np.zeros((4, HD, 512), np.float32)
    cc = np.arange(512)[None, :]
    rr = np.arange(HD)[:, None]
    for j in range(4):
        mask[j] = (cc >= HD * j + rr).astype(np.float32)
    mask = mask.astype(bf)

    xT = [np.ascontiguousarray(x[b].T).astype(np.float16) for b in range(2)]
    in_maps = []
    for c in range(8):
        b, g = divmod(c, 4)
        qcols = np.concatenate([(4 * g + h) * HD + perm for h in range(NQH)])
        kcols = g * HD + perm
        vcols = np.arange(g * HD, (g + 1) * HD)
        in_maps.append({
            "xT": xT[b],
            "wq": Wq[:, qcols].astype(np.float16),
            "wk": Wk[:, kcols].astype(np.float16),
            "wv": Wv[:, vcols].astype(np.float16),
            "wo": (Wo[g * 512:(g + 1) * 512, :] * SCALE).astype(bf),
            "ct": ct,
            "st": st,
            "mask": mask,
            "bq": np.ascontiguousarray(
                bq[np.concatenate([(4 * g + h) * HD + perm for h in range(NQH)])]
                .reshape(NQH, HD).T).astype(np.float32),
            "bk": bk[kcols].reshape(HD, 1).astype(np.float32),
        })

    import time
    t0 = time.time()
    res = run_bass_kernel_spmd(_program, in_maps, list(range(8)))
    t1 = time.time()
    LAST_EXEC_NS = res.exec_time_ns
    if LAST_EXEC_NS is None:
        LAST_EXEC_NS = int((t1 - t0) * 1e9)  # wall time incl. H2D (upper bound)

    out = np.zeros((2, T, E), np.float64)
    for c in range(8):
        out[c // 4] += np.asarray(res.results[c]["out"], np.float64)
    # bv folded: after softmax each row sums to 1, scaled by SCALE inside Wo
    obias = np.repeat(bv.astype(np.float64).reshape(4, HD), 4, axis=0).reshape(-1)
    bo_eff = bo.astype(np.float64) + SCALE * (obias @ Wo.astype(np.float64))
    out += bo_eff[None, None, :]
    return out.astype(np.float32)

